# revision 13
# baseline (speedup 1.0000x reference)
"""Causal self-attention (query-axis softmax) for Trainium2, 8 NeuronCores.

Sharding: 8 cores = 4 batches x 2 half-head-groups. Core c handles batch
c//2 and heads (c%2)*6 .. (c%2)*6+5. Each core computes its heads' full
attention plus its partial output projection; the host sums the two
partials per batch and adds b_proj.

Layout strategy per core (T=2048, C=768, 6 heads, hd=64):
  - host passes x[b].T in bf16 so the QKV contraction dim (C) lands on
    SBUF partitions without any on-chip transpose; all weights bf16.
  - Q,K are produced transposed ([head_d, t]) so S^T = K Q^T tiles have
    softmax's query axis on the free dimension; V is produced in [t, d].
  - softmax over q (free axis): no max-subtraction needed (logits are
    O(1) by construction), exp+rowsum fused on ScalarE via accum_out
    over 1024-wide PSUM chunks; normalization folded into V rows
    (GpSimd scales V[k,:] by 1/denom[k] into zero-padded Vs slots).
  - cross-pair software pipeline: the exp stream (ScalarE) is the
    critical resource, so each head-pair's exp window hides the OTHER
    engines' work: pair 0's window absorbs the remaining QKV matmuls
    (pairs 1-2 Q/K + all V), pair i's window absorbs pair i-1's AV
    drain. exp outputs land in one packed SBUF backlog (at_back);
    AV(kt) for the previous pair re-reads it one step before the
    current pair's exp(kt) overwrites it (WAR dep keeps this safe).
  - causal mask: ragged chunk bounds skip fully-masked blocks; diagonal
    128x128 blocks get a precomputed triangular -30000 add.
  - PSUM: pair-0 window runs QKV accumulators (4 banks) + score chunks
    (4); later windows run score chunks (4) + the previous pair's y^T
    accumulator (4); projection uses its own 4 after scores retire.
"""

import os
import sys

sys.path.insert(0, "/opt/trn_rl_repo")

import numpy as np
import ml_dtypes

import concourse.bass as bass
import concourse.mybir as mybir
import concourse.tile as tile
from concourse.bass_utils import run_bass_kernel_spmd

FP32 = mybir.dt.float32
BF16 = mybir.dt.bfloat16

B, T, C, H = 4, 2048, 768, 12
D = 64                  # head dim
NCORES = 8
HPC = H * B // NCORES   # heads per core = 6
E = HPC * D             # qkv slice width per core = 384
CT = C // 128           # c tiles = 6
ET = E // 128           # e tiles = 3
TT = T // 128           # t tiles = 16
QCH = 512               # fp32 matmul moving chunk (PSUM bank limit)
NQC = T // QCH          # 4
BCH = 1024              # exp chunk
NBC = T // BCH          # 2
MASKV = -30000.0
SCALE = 1.0 / 8.0       # 1/sqrt(hd)
Exp = mybir.ActivationFunctionType.Exp

# packed at_back offsets: slot kt holds columns [klo, T) of the exp tile
OFF = [0] * (TT + 1)
for _kt in range(TT):
    OFF[_kt + 1] = OFF[_kt] + (T - 128 * _kt)
TOFF = OFF[TT]          # 17408


def _split_sync_waits(nc):
    """This container's walrus encodes at most one sync wait per
    instruction for several instruction structs; hoist extra waits onto
    same-engine nops placed immediately before the instruction."""
    for f in nc.m.functions:
        for bb in f.blocks:
            new_insts = []
            for inst in bb.instructions:
                si = inst.sync_info
                waits = list(si.on_wait) if si is not None and si.on_wait else []
                if len(waits) > 1:
                    for w in waits[:-1]:
                        nop = mybir.InstNoOp(
                            name=nc.get_next_instruction_name(),
                            engine=inst.engine,
                            sync_info=mybir.SyncInfo(on_wait=[w], on_update=[]),
                            bass_nofuse=True,
                        )
                        nc.register_instruction(nop)
                        new_insts.append(nop)
                    inst.sync_info = mybir.SyncInfo(
                        on_wait=[waits[-1]], on_update=list(si.on_update or [])
                    )
                new_insts.append(inst)
            bb.instructions[:] = new_insts


def _build():
    nc = bass.Bass("TRN2")
    xT = nc.dram_tensor("xT", [NQC, 128, CT, QCH], BF16, kind="ExternalInput")
    wq = nc.dram_tensor("wq", [128, CT, E], BF16, kind="ExternalInput")
    wk = nc.dram_tensor("wk", [128, CT, E], BF16, kind="ExternalInput")
    wv = nc.dram_tensor("wv", [128, CT, E], BF16, kind="ExternalInput")
    bq = nc.dram_tensor("bq", [E], FP32, kind="ExternalInput")
    bk = nc.dram_tensor("bk", [E], FP32, kind="ExternalInput")
    bv = nc.dram_tensor("bv", [E], FP32, kind="ExternalInput")
    wp = nc.dram_tensor("wp", [128, ET, C], BF16, kind="ExternalInput")
    mask = nc.dram_tensor("mask", [128, 128], FP32, kind="ExternalInput")
    out = nc.dram_tensor("out", [T, C], BF16, kind="ExternalOutput")

    with tile.TileContext(nc) as tc:
        with (
            tc.tile_pool(name="wts", bufs=1) as wts,
            tc.tile_pool(name="big", bufs=1) as big,
            tc.tile_pool(name="sm", bufs=4) as sm,
            tc.tile_pool(name="op", bufs=3) as op,
        ):
            # ---- constant loads: few big DMAs (each dma_start has ~1us
            # issue overhead on the sync engine), ordered for earliest
            # compute start ----
            xt_all = wts.tile([128, NQC, CT, QCH], BF16, name="xt_all")
            wq_sb = wts.tile([128, CT, E], BF16)
            wk_sb = wts.tile([128, CT, E], BF16)
            wv_sb = wts.tile([128, CT, E], BF16)
            bq_sb = wts.tile([128, ET], FP32)
            bk_sb = wts.tile([128, ET], FP32)
            # xt chunks 1-3 go out on the ACT/GpSimd DGE queues so they
            # stream in parallel with the sync queue's weight loads.
            nc.sync.dma_start(out=xt_all[:, 0], in_=xT[0])
            nc.scalar.dma_start(out=xt_all[:, 1], in_=xT[1])
            nc.gpsimd.dma_start(out=xt_all[:, 2], in_=xT[2])
            nc.scalar.dma_start(out=xt_all[:, 3], in_=xT[3])
            nc.sync.dma_start(out=wq_sb, in_=wq[:])
            nc.sync.dma_start(out=bq_sb, in_=bq.rearrange("(et p) -> p et", p=128))
            nc.sync.dma_start(out=wk_sb, in_=wk[:])
            nc.sync.dma_start(out=bk_sb, in_=bk.rearrange("(et p) -> p et", p=128))
            nc.sync.dma_start(out=wv_sb, in_=wv[:])
            bv_sb = wts.tile([128, E], FP32)
            nc.sync.dma_start(out=bv_sb, in_=bv[None, :].to_broadcast((128, E)))
            mask_sb = wts.tile([128, 128], FP32)
            nc.sync.dma_start(out=mask_sb, in_=mask[:])
            wp_sb = wts.tile([128, ET, C], BF16)
            nc.sync.dma_start(out=wp_sb, in_=wp[:])

            qt2 = big.tile([128, ET, T], BF16)      # [d-in-pair, pair, t]
            ktp2 = big.tile([128, ET, 2, T], BF16)  # [d(+zero half), pair, head-in-pair, t]
            v_sb = big.tile([128, TT, E], BF16)     # [t-in-tile, ttile, (head,d)]
            y_sb = big.tile([128, ET, T], BF16)     # [hd-in-pair, pair, t]
            at_back = big.tile([128, 2, TOFF], BF16)  # packed exp backlog per hj
            rcp_all = big.tile([128, 2, TT], FP32)  # 1/denom per (hj, kt), current pair
            # zero-fill ktp2 per-pair so pair 0's K bias adds unblock first
            for et in range(ET):
                nc.gpsimd.memset(ktp2[:, et].bitcast(mybir.dt.uint16), 0)
            # two rotating scaled-V buffers (pairs alternate); slot (hj,kt)
            # holds Vs in cols hj*64..hj*64+64, zeros elsewhere (so two
            # heads' AV matmuls can accumulate into one shared y^T psum).
            vs_bufs = [
                big.tile([128, 2, TT, 128], BF16, name=f"vs{i}") for i in range(2)
            ]
            for t_ in vs_bufs:
                nc.gpsimd.memset(t_, 0.0)

            # ---- QKV group emitters ----
            def q_group(psA, et, tci):
                cols = slice(tci * QCH, (tci + 1) * QCH)
                pq = psA.tile([128, QCH], FP32, tag="ps", bufs=4, name="pq")
                for ct in range(CT):
                    nc.tensor.matmul(
                        pq, wq_sb[:, ct, et * 128:(et + 1) * 128],
                        xt_all[:, tci, ct, :],
                        start=(ct == 0), stop=(ct == CT - 1),
                    )
                nc.vector.tensor_scalar_add(qt2[:, et, cols], pq, bq_sb[:, et:et + 1])

            def k_group(psA, et, tci):
                cols = slice(tci * QCH, (tci + 1) * QCH)
                pk = psA.tile([128, QCH], FP32, tag="ps", bufs=4, name="pk")
                for ct in range(CT):
                    nc.tensor.matmul(
                        pk, wk_sb[:, ct, et * 128:(et + 1) * 128],
                        xt_all[:, tci, ct, :],
                        start=(ct == 0), stop=(ct == CT - 1),
                    )
                nc.vector.tensor_scalar_add(
                    ktp2[0:64, et, 0, cols], pk[0:64, :], bk_sb[0:64, et:et + 1]
                )
                nc.vector.tensor_scalar_add(
                    ktp2[64:128, et, 1, cols], pk[64:128, :], bk_sb[64:128, et:et + 1]
                )

            def v_group(psA, tt):
                tci, ttl = tt // 4, tt % 4
                pv = psA.tile([128, QCH], FP32, tag="ps", bufs=4, name="pv")
                for ct in range(CT):
                    nc.tensor.matmul(
                        pv[:, :E], xt_all[:, tci, ct, ttl * 128:(ttl + 1) * 128],
                        wv_sb[:, ct, :],
                        start=(ct == 0), stop=(ct == CT - 1),
                    )
                nc.vector.tensor_add(v_sb[:, tt, :], pv[:, :E], bv_sb)

            # ---- one S^T/exp step; 1/denom lands in rcp_all ----
            # dve_sum: rowsum via a DVE reduce over the exp output instead
            # of the ScalarE accumulator (ScalarE is the critical engine in
            # the steady-state windows; pair 0's window is PE-bound and its
            # DVE is loaded with QKV evacuations, so it keeps ACT accum).
            def s_step(psS, hp, kt, hj):
                klo = 128 * kt
                bc0 = klo // BCH
                dve_sum = hp > 0 and hj == 0
                sums = sm.tile([128, NBC], FP32, tag="sums", bufs=4, name="sums")
                for bc in range(bc0, NBC):
                    blo = max(BCH * bc, klo)
                    s_ps = psS.tile([128, BCH], FP32, tag="s", bufs=2, name="s_ps")
                    for half in range(2):
                        plo = max(blo, BCH * bc + half * QCH)
                        phi = BCH * bc + (half + 1) * QCH
                        if plo >= phi:
                            continue
                        nc.tensor.matmul(
                            s_ps[:, plo - BCH * bc:phi - BCH * bc],
                            ktp2[:, hp, hj, klo:klo + 128],
                            qt2[:, hp, plo:phi],
                            start=True, stop=True,
                        )
                    if bc == bc0:
                        off = klo - BCH * bc
                        nc.vector.tensor_add(
                            s_ps[:, off:off + 128], s_ps[:, off:off + 128], mask_sb
                        )
                    nc.scalar.activation(
                        at_back[:, hj, OFF[kt] + blo - klo:OFF[kt] + BCH * (bc + 1) - klo],
                        s_ps[:, blo - BCH * bc:],
                        Exp, scale=SCALE,
                        accum_out=None if dve_sum else sums[:, bc:bc + 1],
                    )
                if dve_sum:
                    stot = sm.tile([128, 1], FP32, tag="stot", bufs=4, name="stot")
                    nc.vector.reduce_sum(
                        stot, at_back[:, hj, OFF[kt]:OFF[kt] + T - klo],
                        axis=mybir.AxisListType.X,
                    )
                    nc.vector.reciprocal(rcp_all[:, hj, kt:kt + 1], stot)
                elif bc0 == NBC - 1:
                    nc.vector.reciprocal(rcp_all[:, hj, kt:kt + 1], sums[:, bc0:NBC])
                else:
                    stot = sm.tile([128, 1], FP32, tag="stot", bufs=4, name="stot")
                    nc.vector.reduce_sum(stot, sums[:, bc0:NBC], axis=mybir.AxisListType.X)
                    nc.vector.reciprocal(rcp_all[:, hj, kt:kt + 1], stot)

            # ---- scale V rows of the PREVIOUS pair by its 1/denom ----
            def scale_step(hp_prev, kt, hj, vs):
                hl = 2 * hp_prev + hj
                nc.vector.tensor_scalar_mul(
                    vs[:, hj, kt, hj * 64:hj * 64 + 64],
                    v_sb[:, kt, hl * 64:(hl + 1) * 64], rcp_all[:, hj, kt:kt + 1]
                )

            # ---- AV drain for one (kt, hj) of the previous pair ----
            def av_step(yps, kt, hj, vs):
                klo = 128 * kt
                for qc in range(kt // 4, NQC):
                    lo = max(QCH * qc, klo)
                    hi = QCH * qc + QCH
                    nc.tensor.matmul(
                        yps[:, lo:hi], vs[:, hj, kt, :],
                        at_back[:, hj, OFF[kt] + lo - klo:OFF[kt] + hi - klo],
                        start=(kt == 0 and hj == 0),
                        stop=(kt == min(TT - 1, 4 * qc + 3) and hj == 1),
                        skip_group_check=True,
                    )

            with tc.tile_pool(name="psS", bufs=2, space="PSUM") as psS:
                # ---- pair 0 window: S/exp-0 + remaining QKV as filler ----
                with tc.tile_pool(name="psA", bufs=4, space="PSUM") as psA:
                    for tci in range(NQC):
                        q_group(psA, 0, tci)
                    for tci in range(NQC):
                        k_group(psA, 0, tci)
                    v_group(psA, 0)
                    filler = []
                    for et in (1, 2):
                        for tci in range(NQC):
                            filler.append((q_group, et, tci))
                            filler.append((k_group, et, tci))
                    for kt in range(TT):
                        if kt + 1 < TT:
                            v_group(psA, kt + 1)
                        for hj in range(2):
                            s_step(psS, 0, kt, hj)
                        if filler:
                            fn, a1, a2 = filler.pop(0)
                            fn(psA, a1, a2)
                    for fn, a1, a2 in filler:
                        fn(psA, a1, a2)

                # ---- pairs 1,2: S/exp-i + AV drain of pair i-1 ----
                with tc.tile_pool(name="psY", bufs=1, space="PSUM") as psY:
                    for hp in (1, 2):
                        yps = psY.tile([128, T], FP32, tag="y", name="yps")
                        for kt in range(TT):
                            for hj in range(2):
                                scale_step(hp - 1, kt, hj, vs_bufs[(hp - 1) % 2])
                                av_step(yps, kt, hj, vs_bufs[(hp - 1) % 2])
                                s_step(psS, hp, kt, hj)
                        nc.vector.tensor_copy(y_sb[:, hp - 1, :], yps)
                    # ---- tail: AV drain of pair 2 ----
                    yps = psY.tile([128, T], FP32, tag="y", name="yps")
                    for kt in range(TT):
                        for hj in range(2):
                            scale_step(2, kt, hj, vs_bufs[0])
                            av_step(yps, kt, hj, vs_bufs[0])
                    nc.vector.tensor_copy(y_sb[:, 2, :], yps)

            # ---- output projection ----
            with tc.tile_pool(name="psP", bufs=4, space="PSUM") as psP:
                for tt in range(TT):
                    po1 = psP.tile([128, QCH], FP32, tag="ps", bufs=4, name="po1")
                    po2 = psP.tile([128, QCH], FP32, tag="ps", bufs=4, name="po2")
                    for et in range(ET):
                        nc.tensor.matmul(
                            po1, y_sb[:, et, tt * 128:(tt + 1) * 128], wp_sb[:, et, 0:QCH],
                            start=(et == 0), stop=(et == ET - 1),
                        )
                        nc.tensor.matmul(
                            po2[:, :C - QCH], y_sb[:, et, tt * 128:(tt + 1) * 128],
                            wp_sb[:, et, QCH:C],
                            start=(et == 0), stop=(et == ET - 1),
                        )
                    o_sb = op.tile([128, C], BF16, tag="o", bufs=3, name="o_sb")
                    nc.vector.tensor_copy(o_sb[:, 0:QCH], po1)
                    nc.vector.tensor_copy(o_sb[:, QCH:C], po2[:, :C - QCH])
                    nc.sync.dma_start(out=out[tt * 128:(tt + 1) * 128, :], in_=o_sb)

    _split_sync_waits(nc)
    return nc


_nc_cache = {}
last_result = None


def kernel(x, w_attn, b_attn, w_proj, b_proj):
    global last_result
    if "nc" not in _nc_cache:
        _nc_cache["nc"] = _build()
    nc = _nc_cache["nc"]

    bf16 = ml_dtypes.bfloat16
    x = np.asarray(x, dtype=np.float32)
    w_attn = np.asarray(w_attn, dtype=np.float32)
    b_attn = np.asarray(b_attn, dtype=np.float32)
    w_proj = np.asarray(w_proj, dtype=np.float32)
    b_proj = np.asarray(b_proj, dtype=np.float32)

    tri = np.where(
        np.arange(128)[None, :] >= np.arange(128)[:, None], 0.0, MASKV
    ).astype(np.float32)

    in_maps = []
    for core in range(NCORES):
        b = core // 2
        e0 = (core % 2) * E
        xt_host = np.ascontiguousarray(
            x[b].T.reshape(CT, 128, NQC, QCH).transpose(2, 1, 0, 3).astype(bf16)
        )
        def _wblk(w):
            return np.ascontiguousarray(
                w.reshape(CT, 128, E).transpose(1, 0, 2).astype(bf16)
            )
        in_maps.append({
            "xT": xt_host,
            "wq": _wblk(w_attn[:, e0:e0 + E]),
            "wk": _wblk(w_attn[:, C + e0:C + e0 + E]),
            "wv": _wblk(w_attn[:, 2 * C + e0:2 * C + e0 + E]),
            "bq": np.ascontiguousarray(b_attn[e0:e0 + E]),
            "bk": np.ascontiguousarray(b_attn[C + e0:C + e0 + E]),
            "bv": np.ascontiguousarray(b_attn[2 * C + e0:2 * C + e0 + E]),
            "wp": np.ascontiguousarray(
                w_proj[e0:e0 + E, :].reshape(ET, 128, C).transpose(1, 0, 2).astype(bf16)
            ),
            "mask": tri,
        })

    trace = os.environ.get("ATT_TRACE", "0")
    kw = {}
    if trace != "0":
        n = min(int(trace), NCORES)
        kw = dict(trace=True, trace_cores=list(range(n)))
    res = run_bass_kernel_spmd(nc, in_maps, list(range(NCORES)), **kw)
    last_result = res

    out = np.zeros((B, T, C), dtype=np.float32)
    for core in range(NCORES):
        out[core // 2] += res.results[core]["out"].astype(np.float32)
    out += b_proj[None, None, :]
    return out


# revision 19
# speedup vs baseline: 1.0894x; 1.0894x over previous
"""Causal self-attention (query-axis softmax) for Trainium2, 8 NeuronCores.

Sharding: 8 cores = 4 batches x 2 half-head-groups. Core c handles batch
c//2 and heads (c%2)*6 .. (c%2)*6+5. Each core computes its heads' full
attention plus its partial output projection; the host sums the two
partials per batch and adds b_proj.

Layout strategy per core (T=2048, C=768, 6 heads, hd=64):
  - host passes x[b].T in bf16 so the QKV contraction dim (C) lands on
    SBUF partitions without any on-chip transpose; all weights bf16.
  - Q,K are produced transposed ([head_d, t]) so S^T = K Q^T tiles have
    softmax's query axis on the free dimension; V is produced in [t, d].
  - softmax over q (free axis): no max-subtraction needed (logits are
    O(1) by construction), exp+rowsum fused on ScalarE via accum_out
    over 1024-wide PSUM chunks; normalization folded into V rows
    (GpSimd scales V[k,:] by 1/denom[k] into zero-padded Vs slots).
  - cross-pair software pipeline: the exp stream (ScalarE) is the
    critical resource, so each head-pair's exp window hides the OTHER
    engines' work: pair 0's window absorbs the remaining QKV matmuls
    (pairs 1-2 Q/K + all V), pair i's window absorbs pair i-1's AV
    drain. exp outputs land in one packed SBUF backlog (at_back);
    AV(kt) for the previous pair re-reads it one step before the
    current pair's exp(kt) overwrites it (WAR dep keeps this safe).
  - causal mask: ragged chunk bounds skip fully-masked blocks; diagonal
    128x128 blocks get a precomputed triangular -30000 add.
  - PSUM: pair-0 window runs QKV accumulators (4 banks) + score chunks
    (4); later windows run score chunks (4) + the previous pair's y^T
    accumulator (4); projection uses its own 4 after scores retire.
"""

import os
import sys

sys.path.insert(0, "/opt/trn_rl_repo")

import numpy as np
import ml_dtypes

import concourse.bass as bass
import concourse.mybir as mybir
import concourse.tile as tile
from concourse.bass_utils import run_bass_kernel_spmd

FP32 = mybir.dt.float32
BF16 = mybir.dt.bfloat16

B, T, C, H = 4, 2048, 768, 12
D = 64                  # head dim
NCORES = 8
HPC = H * B // NCORES   # heads per core = 6
E = HPC * D             # qkv slice width per core = 384
CT = C // 128           # c tiles = 6
ET = E // 128           # e tiles = 3
TT = T // 128           # t tiles = 16
QCH = 512               # fp32 matmul moving chunk (PSUM bank limit)
NQC = T // QCH          # 4
BCH = 1024              # exp chunk
NBC = T // BCH          # 2
MASKV = -30000.0
SCALE = 1.0 / 8.0       # 1/sqrt(hd)
Exp = mybir.ActivationFunctionType.Exp

# packed at_back offsets: slot kt holds columns [klo, T) of the exp tile
OFF = [0] * (TT + 1)
for _kt in range(TT):
    OFF[_kt + 1] = OFF[_kt] + (T - 128 * _kt)
TOFF = OFF[TT]          # 17408


def _split_sync_waits(nc):
    """This container's walrus encodes at most one sync wait per
    instruction for several instruction structs; hoist extra waits onto
    same-engine nops placed immediately before the instruction."""
    for f in nc.m.functions:
        for bb in f.blocks:
            new_insts = []
            for inst in bb.instructions:
                si = inst.sync_info
                waits = list(si.on_wait) if si is not None and si.on_wait else []
                if len(waits) > 1:
                    for w in waits[:-1]:
                        nop = mybir.InstNoOp(
                            name=nc.get_next_instruction_name(),
                            engine=inst.engine,
                            sync_info=mybir.SyncInfo(on_wait=[w], on_update=[]),
                            bass_nofuse=True,
                        )
                        nc.register_instruction(nop)
                        new_insts.append(nop)
                    inst.sync_info = mybir.SyncInfo(
                        on_wait=[waits[-1]], on_update=list(si.on_update or [])
                    )
                new_insts.append(inst)
            bb.instructions[:] = new_insts


def _build():
    nc = bass.Bass("TRN2")
    xT = nc.dram_tensor("xT", [NQC, 128, CT, QCH], BF16, kind="ExternalInput")
    wq = nc.dram_tensor("wq", [128, CT, E], BF16, kind="ExternalInput")
    wk = nc.dram_tensor("wk", [128, CT, E], BF16, kind="ExternalInput")
    wv = nc.dram_tensor("wv", [128, CT, E], BF16, kind="ExternalInput")
    bq = nc.dram_tensor("bq", [E], FP32, kind="ExternalInput")
    bk = nc.dram_tensor("bk", [E], FP32, kind="ExternalInput")
    bv = nc.dram_tensor("bv", [E], FP32, kind="ExternalInput")
    wp = nc.dram_tensor("wp", [128, ET, C], BF16, kind="ExternalInput")
    mask = nc.dram_tensor("mask", [128, 128], FP32, kind="ExternalInput")
    out = nc.dram_tensor("out", [T, C], BF16, kind="ExternalOutput")

    with tile.TileContext(nc) as tc:
        with (
            tc.tile_pool(name="wts", bufs=1) as wts,
            tc.tile_pool(name="big", bufs=1) as big,
            tc.tile_pool(name="sm", bufs=4) as sm,
            tc.tile_pool(name="op", bufs=3) as op,
        ):
            # ---- constant loads: few big DMAs (each dma_start has ~1us
            # issue overhead on the sync engine), ordered for earliest
            # compute start ----
            xt_all = wts.tile([128, NQC, CT, QCH], BF16, name="xt_all")
            wq_sb = wts.tile([128, CT, E], BF16)
            wk_sb = wts.tile([128, CT, E], BF16)
            wv_sb = wts.tile([128, CT, E], BF16)
            bq_sb = wts.tile([128, ET], FP32)
            bk_sb = wts.tile([128, ET], FP32)
            mask_sb = wts.tile([128, 128], FP32)
            nc.sync.dma_start(out=xt_all[:, 0], in_=xT[0])
            nc.sync.dma_start(out=wq_sb, in_=wq[:])
            nc.sync.dma_start(out=bq_sb, in_=bq.rearrange("(et p) -> p et", p=128))
            nc.sync.dma_start(out=mask_sb, in_=mask[:])
            nc.sync.dma_start(out=wk_sb, in_=wk[:])
            nc.sync.dma_start(out=bk_sb, in_=bk.rearrange("(et p) -> p et", p=128))
            nc.sync.dma_start(out=xt_all[:, 1], in_=xT[1])
            nc.sync.dma_start(out=xt_all[:, 2], in_=xT[2])
            nc.sync.dma_start(out=xt_all[:, 3], in_=xT[3])
            nc.sync.dma_start(out=wv_sb, in_=wv[:])
            bv_sb = wts.tile([128, E], FP32)
            nc.sync.dma_start(out=bv_sb, in_=bv[None, :].to_broadcast((128, E)))
            wp_sb = wts.tile([128, ET, C], BF16)
            nc.sync.dma_start(out=wp_sb, in_=wp[:])

            qt2 = big.tile([128, ET, T], BF16)      # [d-in-pair, pair, t]
            ktp2 = big.tile([128, ET, 2, T], BF16)  # [d(+zero half), pair, head-in-pair, t]
            v_sb = big.tile([128, TT, E], BF16)     # [t-in-tile, ttile, (head,d)]
            y_sb = big.tile([128, ET, T], BF16)     # [hd-in-pair, pair, t]
            at_back = big.tile([128, 2, TOFF], BF16)  # packed exp backlog per hj
            rcp_all = big.tile([128, 2, TT], FP32)  # 1/denom per (hj, kt), current pair
            # zero-fill ktp2 per-pair on DVE (gpsimd's startup preamble is
            # ~10us; pair 0's K bias adds must unblock early)
            for et in range(ET):
                nc.vector.memset(ktp2[:, et].bitcast(mybir.dt.uint16), 0)
            # two rotating scaled-V buffers (pairs alternate); slot (hj,kt)
            # holds Vs in cols hj*64..hj*64+64, zeros elsewhere (so two
            # heads' AV matmuls can accumulate into one shared y^T psum).
            vs_bufs = [
                big.tile([128, 2, TT, 128], BF16, name=f"vs{i}") for i in range(2)
            ]
            for t_ in vs_bufs:
                nc.gpsimd.memset(t_, 0.0)

            # ---- QKV group emitters ----
            def q_group(psA, et, tci):
                cols = slice(tci * QCH, (tci + 1) * QCH)
                pq = psA.tile([128, QCH], FP32, tag="ps", bufs=4, name="pq")
                for ct in range(CT):
                    nc.tensor.matmul(
                        pq, wq_sb[:, ct, et * 128:(et + 1) * 128],
                        xt_all[:, tci, ct, :],
                        start=(ct == 0), stop=(ct == CT - 1),
                    )
                nc.vector.tensor_scalar_add(qt2[:, et, cols], pq, bq_sb[:, et:et + 1])

            def k_group(psA, et, tci):
                cols = slice(tci * QCH, (tci + 1) * QCH)
                pk = psA.tile([128, QCH], FP32, tag="ps", bufs=4, name="pk")
                for ct in range(CT):
                    nc.tensor.matmul(
                        pk, wk_sb[:, ct, et * 128:(et + 1) * 128],
                        xt_all[:, tci, ct, :],
                        start=(ct == 0), stop=(ct == CT - 1),
                    )
                nc.vector.tensor_scalar_add(
                    ktp2[0:64, et, 0, cols], pk[0:64, :], bk_sb[0:64, et:et + 1]
                )
                nc.vector.tensor_scalar_add(
                    ktp2[64:128, et, 1, cols], pk[64:128, :], bk_sb[64:128, et:et + 1]
                )

            def v_group(psA, tt):
                tci, ttl = tt // 4, tt % 4
                pv = psA.tile([128, QCH], FP32, tag="ps", bufs=4, name="pv")
                for ct in range(CT):
                    nc.tensor.matmul(
                        pv[:, :E], xt_all[:, tci, ct, ttl * 128:(ttl + 1) * 128],
                        wv_sb[:, ct, :],
                        start=(ct == 0), stop=(ct == CT - 1),
                    )
                nc.vector.tensor_add(v_sb[:, tt, :], pv[:, :E], bv_sb)

            # ---- one S^T/exp step; 1/denom lands in rcp_all ----
            # dve_sum: rowsum via a DVE reduce over the exp output instead
            # of the ScalarE accumulator (ScalarE is the critical engine in
            # the steady-state windows; pair 0's window is PE-bound and its
            # DVE is loaded with QKV evacuations, so it keeps ACT accum).
            def s_step(psS, hp, kt, hj):
                klo = 128 * kt
                bc0 = klo // BCH
                dve_sum = False  # DVE reduce costs more than ACT accum saves
                sums = sm.tile([128, NBC], FP32, tag="sums", bufs=4, name="sums")
                for bc in range(bc0, NBC):
                    blo = max(BCH * bc, klo)
                    s_ps = psS.tile([128, BCH], FP32, tag="s", bufs=2, name="s_ps")
                    for half in range(2):
                        plo = max(blo, BCH * bc + half * QCH)
                        phi = BCH * bc + (half + 1) * QCH
                        if plo >= phi:
                            continue
                        nc.tensor.matmul(
                            s_ps[:, plo - BCH * bc:phi - BCH * bc],
                            ktp2[:, hp, hj, klo:klo + 128],
                            qt2[:, hp, plo:phi],
                            start=True, stop=True,
                        )
                    if bc == bc0:
                        off = klo - BCH * bc
                        nc.vector.tensor_add(
                            s_ps[:, off:off + 128], s_ps[:, off:off + 128], mask_sb
                        )
                    nc.scalar.activation(
                        at_back[:, hj, OFF[kt] + blo - klo:OFF[kt] + BCH * (bc + 1) - klo],
                        s_ps[:, blo - BCH * bc:],
                        Exp, scale=SCALE,
                        accum_out=None if dve_sum else sums[:, bc:bc + 1],
                    )
                if dve_sum:
                    stot = sm.tile([128, 1], FP32, tag="stot", bufs=4, name="stot")
                    nc.vector.reduce_sum(
                        stot, at_back[:, hj, OFF[kt]:OFF[kt] + T - klo],
                        axis=mybir.AxisListType.X,
                    )
                    nc.vector.reciprocal(rcp_all[:, hj, kt:kt + 1], stot)
                elif bc0 == NBC - 1:
                    nc.vector.reciprocal(rcp_all[:, hj, kt:kt + 1], sums[:, bc0:NBC])
                else:
                    stot = sm.tile([128, 1], FP32, tag="stot", bufs=4, name="stot")
                    nc.vector.reduce_sum(stot, sums[:, bc0:NBC], axis=mybir.AxisListType.X)
                    nc.vector.reciprocal(rcp_all[:, hj, kt:kt + 1], stot)

            # ---- scale V rows of the PREVIOUS pair by its 1/denom ----
            def scale_step(hp_prev, kt, hj, vs):
                hl = 2 * hp_prev + hj
                nc.vector.tensor_scalar_mul(
                    vs[:, hj, kt, hj * 64:hj * 64 + 64],
                    v_sb[:, kt, hl * 64:(hl + 1) * 64], rcp_all[:, hj, kt:kt + 1]
                )

            # ---- AV drain for one (kt, hj) of the previous pair; y^T is
            # held as four 512-wide 1-bank tiles so finished q-chunks can
            # release their bank mid-window ----
            def av_step(ytiles, kt, hj, vs):
                klo = 128 * kt
                for qc in range(kt // 4, NQC):
                    lo = max(QCH * qc, klo)
                    hi = QCH * qc + QCH
                    nc.tensor.matmul(
                        ytiles[qc][:, lo - QCH * qc:hi - QCH * qc], vs[:, hj, kt, :],
                        at_back[:, hj, OFF[kt] + lo - klo:OFF[kt] + hi - klo],
                        start=(kt == 0 and hj == 0),
                        stop=(kt == min(TT - 1, 4 * qc + 3) and hj == 1),
                        skip_group_check=True,
                    )

            # ---- all contributions to q-chunk j of the LAST pair ----
            def av2_qc(ytile, j, vs):
                last_kt = min(TT - 1, 4 * j + 3)
                for kt in range(last_kt + 1):
                    klo = 128 * kt
                    lo = max(QCH * j, klo)
                    hi = QCH * j + QCH
                    for hj in range(2):
                        nc.tensor.matmul(
                            ytile[:, lo - QCH * j:hi - QCH * j], vs[:, hj, kt, :],
                            at_back[:, hj, OFF[kt] + lo - klo:OFF[kt] + hi - klo],
                            start=(kt == 0 and hj == 0),
                            stop=(kt == last_kt and hj == 1),
                            skip_group_check=True,
                        )

            with tc.tile_pool(name="psS", bufs=2, space="PSUM") as psS:
                # ---- pair 0 window: S/exp-0 + remaining QKV as filler ----
                with tc.tile_pool(name="psA", bufs=4, space="PSUM") as psA:
                    for tci in range(NQC):
                        q_group(psA, 0, tci)
                    for tci in range(NQC):
                        k_group(psA, 0, tci)
                    v_group(psA, 0)
                    filler = []
                    for et in (1, 2):
                        for tci in range(NQC):
                            filler.append((q_group, et, tci))
                            filler.append((k_group, et, tci))
                    for kt in range(TT):
                        if kt + 1 < TT:
                            v_group(psA, kt + 1)
                        for hj in range(2):
                            s_step(psS, 0, kt, hj)
                        if filler:
                            fn, a1, a2 = filler.pop(0)
                            fn(psA, a1, a2)
                    for fn, a1, a2 in filler:
                        fn(psA, a1, a2)

                # ---- pairs 1,2: S/exp-i + AV drain of pair i-1 ----
                with tc.tile_pool(name="psY", bufs=4, space="PSUM") as psY:
                    def ytile(name):
                        return psY.tile([128, QCH], FP32, tag="y", bufs=4, name=name)

                    # window 1: drain pair 0
                    y0 = [ytile(f"y0_{j}") for j in range(NQC)]
                    for kt in range(TT):
                        for hj in range(2):
                            scale_step(0, kt, hj, vs_bufs[0])
                            av_step(y0, kt, hj, vs_bufs[0])
                            s_step(psS, 1, kt, hj)
                    for j in range(NQC):
                        nc.vector.tensor_copy(y_sb[:, 0, QCH * j:QCH * (j + 1)], y0[j])
                    # window 2: drain pair 1; as each of pair 1's q-chunks
                    # finalizes, recycle its bank for pair 2's same q-chunk
                    # (its exps are complete by then), leaving only q-chunk 3
                    # of pair 2 plus the projection for the tail.
                    y1 = [ytile(f"y1_{j}") for j in range(NQC)]
                    y2 = [None] * NQC
                    for kt in range(TT):
                        for hj in range(2):
                            scale_step(1, kt, hj, vs_bufs[1])
                            av_step(y1, kt, hj, vs_bufs[1])
                            s_step(psS, 2, kt, hj)
                            scale_step(2, kt, hj, vs_bufs[0])
                        if kt % 4 == 3 and kt < TT - 1:
                            j = kt // 4
                            nc.vector.tensor_copy(y_sb[:, 1, QCH * j:QCH * (j + 1)], y1[j])
                            y2[j] = ytile(f"y2_{j}")
                            av2_qc(y2[j], j, vs_bufs[0])
                            nc.vector.tensor_copy(y_sb[:, 2, QCH * j:QCH * (j + 1)], y2[j])
                    nc.vector.tensor_copy(y_sb[:, 1, QCH * 3:], y1[3])
                    # ---- tail: last q-chunk of pair 2, then projection ----
                    y2[3] = ytile("y2_3")
                    av2_qc(y2[3], 3, vs_bufs[0])
                    nc.vector.tensor_copy(y_sb[:, 2, QCH * 3:], y2[3])

                    # ---- output projection (reuses the y bank pool) ----
                    for tt in range(TT):
                        po1 = ytile("po1")
                        po2 = ytile("po2")
                        for et in range(ET):
                            nc.tensor.matmul(
                                po1, y_sb[:, et, tt * 128:(tt + 1) * 128],
                                wp_sb[:, et, 0:QCH],
                                start=(et == 0), stop=(et == ET - 1),
                            )
                            nc.tensor.matmul(
                                po2[:, :C - QCH], y_sb[:, et, tt * 128:(tt + 1) * 128],
                                wp_sb[:, et, QCH:C],
                                start=(et == 0), stop=(et == ET - 1),
                            )
                        o_sb = op.tile([128, C], BF16, tag="o", bufs=3, name="o_sb")
                        nc.vector.tensor_copy(o_sb[:, 0:QCH], po1)
                        nc.vector.tensor_copy(o_sb[:, QCH:C], po2[:, :C - QCH])
                        nc.sync.dma_start(out=out[tt * 128:(tt + 1) * 128, :], in_=o_sb)

    _split_sync_waits(nc)
    return nc


_nc_cache = {}
last_result = None


def kernel(x, w_attn, b_attn, w_proj, b_proj):
    global last_result
    if "nc" not in _nc_cache:
        _nc_cache["nc"] = _build()
    nc = _nc_cache["nc"]

    bf16 = ml_dtypes.bfloat16
    x = np.asarray(x, dtype=np.float32)
    w_attn = np.asarray(w_attn, dtype=np.float32)
    b_attn = np.asarray(b_attn, dtype=np.float32)
    w_proj = np.asarray(w_proj, dtype=np.float32)
    b_proj = np.asarray(b_proj, dtype=np.float32)

    tri = np.where(
        np.arange(128)[None, :] >= np.arange(128)[:, None], 0.0, MASKV
    ).astype(np.float32)

    in_maps = []
    for core in range(NCORES):
        b = core // 2
        e0 = (core % 2) * E
        xt_host = np.ascontiguousarray(
            x[b].T.reshape(CT, 128, NQC, QCH).transpose(2, 1, 0, 3).astype(bf16)
        )
        def _wblk(w):
            return np.ascontiguousarray(
                w.reshape(CT, 128, E).transpose(1, 0, 2).astype(bf16)
            )
        in_maps.append({
            "xT": xt_host,
            "wq": _wblk(w_attn[:, e0:e0 + E]),
            "wk": _wblk(w_attn[:, C + e0:C + e0 + E]),
            "wv": _wblk(w_attn[:, 2 * C + e0:2 * C + e0 + E]),
            "bq": np.ascontiguousarray(b_attn[e0:e0 + E]),
            "bk": np.ascontiguousarray(b_attn[C + e0:C + e0 + E]),
            "bv": np.ascontiguousarray(b_attn[2 * C + e0:2 * C + e0 + E]),
            "wp": np.ascontiguousarray(
                w_proj[e0:e0 + E, :].reshape(ET, 128, C).transpose(1, 0, 2).astype(bf16)
            ),
            "mask": tri,
        })

    trace = os.environ.get("ATT_TRACE", "0")
    kw = {}
    if trace != "0":
        n = min(int(trace), NCORES)
        kw = dict(trace=True, trace_cores=list(range(n)))
    res = run_bass_kernel_spmd(nc, in_maps, list(range(NCORES)), **kw)
    last_result = res

    out = np.zeros((B, T, C), dtype=np.float32)
    for core in range(NCORES):
        out[core // 2] += res.results[core]["out"].astype(np.float32)
    out += b_proj[None, None, :]
    return out


# revision 21
# speedup vs baseline: 1.0942x; 1.0044x over previous
"""Causal self-attention (query-axis softmax) for Trainium2, 8 NeuronCores.

Sharding: 8 cores = 4 batches x 2 half-head-groups. Core c handles batch
c//2 and heads (c%2)*6 .. (c%2)*6+5. Each core computes its heads' full
attention plus its partial output projection; the host sums the two
partials per batch and adds b_proj.

Layout strategy per core (T=2048, C=768, 6 heads, hd=64):
  - host passes x[b].T in bf16 so the QKV contraction dim (C) lands on
    SBUF partitions without any on-chip transpose; all weights bf16.
  - Q,K are produced transposed ([head_d, t]) so S^T = K Q^T tiles have
    softmax's query axis on the free dimension; V is produced in [t, d].
  - softmax over q (free axis): no max-subtraction needed (logits are
    O(1) by construction), exp+rowsum fused on ScalarE via accum_out
    over 1024-wide PSUM chunks; normalization folded into V rows
    (GpSimd scales V[k,:] by 1/denom[k] into zero-padded Vs slots).
  - cross-pair software pipeline: the exp stream (ScalarE) is the
    critical resource, so each head-pair's exp window hides the OTHER
    engines' work: pair 0's window absorbs the remaining QKV matmuls
    (pairs 1-2 Q/K + all V), pair i's window absorbs pair i-1's AV
    drain. exp outputs land in one packed SBUF backlog (at_back);
    AV(kt) for the previous pair re-reads it one step before the
    current pair's exp(kt) overwrites it (WAR dep keeps this safe).
  - causal mask: ragged chunk bounds skip fully-masked blocks; diagonal
    128x128 blocks get a precomputed triangular -30000 add.
  - PSUM: pair-0 window runs QKV accumulators (4 banks) + score chunks
    (4); later windows run score chunks (4) + the previous pair's y^T
    accumulator (4); projection uses its own 4 after scores retire.
"""

import os
import sys

sys.path.insert(0, "/opt/trn_rl_repo")

import numpy as np
import ml_dtypes

import concourse.bass as bass
import concourse.mybir as mybir
import concourse.tile as tile
from concourse.bass_utils import run_bass_kernel_spmd

FP32 = mybir.dt.float32
BF16 = mybir.dt.bfloat16

B, T, C, H = 4, 2048, 768, 12
D = 64                  # head dim
NCORES = 8
HPC = H * B // NCORES   # heads per core = 6
E = HPC * D             # qkv slice width per core = 384
CT = C // 128           # c tiles = 6
ET = E // 128           # e tiles = 3
TT = T // 128           # t tiles = 16
QCH = 512               # fp32 matmul moving chunk (PSUM bank limit)
NQC = T // QCH          # 4
BCH = 1024              # exp chunk
NBC = T // BCH          # 2
MASKV = -30000.0
SCALE = 1.0 / 8.0       # 1/sqrt(hd)
Exp = mybir.ActivationFunctionType.Exp

# packed at_back offsets: slot kt holds columns [klo, T) of the exp tile
OFF = [0] * (TT + 1)
for _kt in range(TT):
    OFF[_kt + 1] = OFF[_kt] + (T - 128 * _kt)
TOFF = OFF[TT]          # 17408


def _split_sync_waits(nc):
    """This container's walrus encodes at most one sync wait per
    instruction for several instruction structs; hoist extra waits onto
    same-engine nops placed immediately before the instruction."""
    for f in nc.m.functions:
        for bb in f.blocks:
            new_insts = []
            for inst in bb.instructions:
                si = inst.sync_info
                waits = list(si.on_wait) if si is not None and si.on_wait else []
                if len(waits) > 1:
                    for w in waits[:-1]:
                        nop = mybir.InstNoOp(
                            name=nc.get_next_instruction_name(),
                            engine=inst.engine,
                            sync_info=mybir.SyncInfo(on_wait=[w], on_update=[]),
                            bass_nofuse=True,
                        )
                        nc.register_instruction(nop)
                        new_insts.append(nop)
                    inst.sync_info = mybir.SyncInfo(
                        on_wait=[waits[-1]], on_update=list(si.on_update or [])
                    )
                new_insts.append(inst)
            bb.instructions[:] = new_insts


def _build():
    nc = bass.Bass("TRN2")
    xT = nc.dram_tensor("xT", [NQC, 128, CT, QCH], BF16, kind="ExternalInput")
    wq = nc.dram_tensor("wq", [128, CT, E], BF16, kind="ExternalInput")
    wk = nc.dram_tensor("wk", [128, CT, E], BF16, kind="ExternalInput")
    wv = nc.dram_tensor("wv", [128, CT, E], BF16, kind="ExternalInput")
    bq = nc.dram_tensor("bq", [E], FP32, kind="ExternalInput")
    bk = nc.dram_tensor("bk", [E], FP32, kind="ExternalInput")
    bv = nc.dram_tensor("bv", [E], FP32, kind="ExternalInput")
    wp = nc.dram_tensor("wp", [128, ET, C], BF16, kind="ExternalInput")
    mask = nc.dram_tensor("mask", [128, 128], FP32, kind="ExternalInput")
    out = nc.dram_tensor("out", [T, C], BF16, kind="ExternalOutput")

    with tile.TileContext(nc) as tc:
        with (
            tc.tile_pool(name="wts", bufs=1) as wts,
            tc.tile_pool(name="big", bufs=1) as big,
            tc.tile_pool(name="sm", bufs=4) as sm,
            tc.tile_pool(name="op", bufs=3) as op,
        ):
            # ---- constant loads: few big DMAs (each dma_start has ~1us
            # issue overhead on the sync engine), ordered for earliest
            # compute start ----
            xt_all = wts.tile([128, NQC, CT, QCH], BF16, name="xt_all")
            wq_sb = wts.tile([128, CT, E], BF16)
            wk_sb = wts.tile([128, CT, E], BF16)
            wv_sb = wts.tile([128, CT, E], BF16)
            bq_sb = wts.tile([128, ET], FP32)
            bk_sb = wts.tile([128, ET], FP32)
            mask_sb = wts.tile([128, 128], FP32)
            nc.sync.dma_start(out=xt_all[:, 0], in_=xT[0])
            nc.sync.dma_start(out=wq_sb, in_=wq[:])
            nc.sync.dma_start(out=bq_sb, in_=bq.rearrange("(et p) -> p et", p=128))
            nc.sync.dma_start(out=mask_sb, in_=mask[:])
            nc.sync.dma_start(out=wk_sb, in_=wk[:])
            nc.sync.dma_start(out=bk_sb, in_=bk.rearrange("(et p) -> p et", p=128))
            nc.sync.dma_start(out=xt_all[:, 1], in_=xT[1])
            nc.sync.dma_start(out=xt_all[:, 2], in_=xT[2])
            nc.sync.dma_start(out=xt_all[:, 3], in_=xT[3])
            nc.sync.dma_start(out=wv_sb, in_=wv[:])
            bv_sb = wts.tile([128, E], FP32)
            nc.sync.dma_start(out=bv_sb, in_=bv[None, :].to_broadcast((128, E)))
            wp_sb = wts.tile([128, ET, C], BF16)
            nc.sync.dma_start(out=wp_sb, in_=wp[:])

            qt2 = big.tile([128, ET, T], BF16)      # [d-in-pair, pair, t]
            ktp2 = big.tile([128, ET, 2, T], BF16)  # [d(+zero half), pair, head-in-pair, t]
            v_sb = big.tile([128, TT, E], BF16)     # [t-in-tile, ttile, (head,d)]
            y_sb = big.tile([128, ET, T], BF16)     # [hd-in-pair, pair, t]
            at_back = big.tile([128, 2, TOFF], BF16)  # packed exp backlog per hj
            rcp_all = big.tile([128, 2, TT], FP32)  # 1/denom per (hj, kt), current pair
            # zero-fill ktp2 per-pair on DVE (gpsimd's startup preamble is
            # ~10us; pair 0's K bias adds must unblock early)
            for et in range(ET):
                nc.vector.memset(ktp2[:, et].bitcast(mybir.dt.uint16), 0)
            # two rotating scaled-V buffers (pairs alternate); slot (hj,kt)
            # holds Vs in cols hj*64..hj*64+64, zeros elsewhere (so two
            # heads' AV matmuls can accumulate into one shared y^T psum).
            vs_bufs = [
                big.tile([128, 2, TT, 128], BF16, name=f"vs{i}") for i in range(2)
            ]
            for t_ in vs_bufs:
                nc.gpsimd.memset(t_, 0.0)

            # ---- QKV group emitters ----
            def q_group(psA, et, tci):
                cols = slice(tci * QCH, (tci + 1) * QCH)
                pq = psA.tile([128, QCH], FP32, tag="ps", bufs=4, name="pq")
                for ct in range(CT):
                    nc.tensor.matmul(
                        pq, wq_sb[:, ct, et * 128:(et + 1) * 128],
                        xt_all[:, tci, ct, :],
                        start=(ct == 0), stop=(ct == CT - 1),
                    )
                nc.vector.tensor_scalar_add(qt2[:, et, cols], pq, bq_sb[:, et:et + 1])

            def k_group(psA, et, tci):
                cols = slice(tci * QCH, (tci + 1) * QCH)
                pk = psA.tile([128, QCH], FP32, tag="ps", bufs=4, name="pk")
                for ct in range(CT):
                    nc.tensor.matmul(
                        pk, wk_sb[:, ct, et * 128:(et + 1) * 128],
                        xt_all[:, tci, ct, :],
                        start=(ct == 0), stop=(ct == CT - 1),
                    )
                nc.vector.tensor_scalar_add(
                    ktp2[0:64, et, 0, cols], pk[0:64, :], bk_sb[0:64, et:et + 1]
                )
                nc.vector.tensor_scalar_add(
                    ktp2[64:128, et, 1, cols], pk[64:128, :], bk_sb[64:128, et:et + 1]
                )

            def v_group(psA, tt):
                tci, ttl = tt // 4, tt % 4
                pv = psA.tile([128, QCH], FP32, tag="ps", bufs=4, name="pv")
                for ct in range(CT):
                    nc.tensor.matmul(
                        pv[:, :E], xt_all[:, tci, ct, ttl * 128:(ttl + 1) * 128],
                        wv_sb[:, ct, :],
                        start=(ct == 0), stop=(ct == CT - 1),
                    )
                nc.vector.tensor_add(v_sb[:, tt, :], pv[:, :E], bv_sb)

            # ---- one S^T/exp step; 1/denom lands in rcp_all ----
            # dve_sum: rowsum via a DVE reduce over the exp output instead
            # of the ScalarE accumulator (ScalarE is the critical engine in
            # the steady-state windows; pair 0's window is PE-bound and its
            # DVE is loaded with QKV evacuations, so it keeps ACT accum).
            def s_step(psS, hp, kt, hj):
                klo = 128 * kt
                bc0 = klo // BCH
                dve_sum = False  # DVE reduce costs more than ACT accum saves
                sums = sm.tile([128, NBC], FP32, tag="sums", bufs=4, name="sums")
                for bc in range(bc0, NBC):
                    blo = max(BCH * bc, klo)
                    s_ps = psS.tile([128, BCH], FP32, tag="s", bufs=2, name="s_ps")
                    for half in range(2):
                        plo = max(blo, BCH * bc + half * QCH)
                        phi = BCH * bc + (half + 1) * QCH
                        if plo >= phi:
                            continue
                        nc.tensor.matmul(
                            s_ps[:, plo - BCH * bc:phi - BCH * bc],
                            ktp2[:, hp, hj, klo:klo + 128],
                            qt2[:, hp, plo:phi],
                            start=True, stop=True,
                        )
                    if bc == bc0:
                        off = klo - BCH * bc
                        nc.vector.tensor_add(
                            s_ps[:, off:off + 128], s_ps[:, off:off + 128], mask_sb
                        )
                    nc.scalar.activation(
                        at_back[:, hj, OFF[kt] + blo - klo:OFF[kt] + BCH * (bc + 1) - klo],
                        s_ps[:, blo - BCH * bc:],
                        Exp, scale=SCALE,
                        accum_out=None if dve_sum else sums[:, bc:bc + 1],
                    )
                if dve_sum:
                    stot = sm.tile([128, 1], FP32, tag="stot", bufs=4, name="stot")
                    nc.vector.reduce_sum(
                        stot, at_back[:, hj, OFF[kt]:OFF[kt] + T - klo],
                        axis=mybir.AxisListType.X,
                    )
                    nc.vector.reciprocal(rcp_all[:, hj, kt:kt + 1], stot)
                elif bc0 == NBC - 1:
                    nc.vector.reciprocal(rcp_all[:, hj, kt:kt + 1], sums[:, bc0:NBC])
                else:
                    stot = sm.tile([128, 1], FP32, tag="stot", bufs=4, name="stot")
                    nc.vector.reduce_sum(stot, sums[:, bc0:NBC], axis=mybir.AxisListType.X)
                    nc.vector.reciprocal(rcp_all[:, hj, kt:kt + 1], stot)

            # ---- scale V rows of the PREVIOUS pair by its 1/denom ----
            def scale_step(hp_prev, kt, hj, vs):
                hl = 2 * hp_prev + hj
                nc.vector.tensor_scalar_mul(
                    vs[:, hj, kt, hj * 64:hj * 64 + 64],
                    v_sb[:, kt, hl * 64:(hl + 1) * 64], rcp_all[:, hj, kt:kt + 1]
                )

            # ---- AV drain for one (kt, hj) of the previous pair; y^T is
            # held as four 512-wide 1-bank tiles so finished q-chunks can
            # release their bank mid-window ----
            def av_step(ytiles, kt, hj, vs):
                klo = 128 * kt
                for qc in range(kt // 4, NQC):
                    lo = max(QCH * qc, klo)
                    hi = QCH * qc + QCH
                    nc.tensor.matmul(
                        ytiles[qc][:, lo - QCH * qc:hi - QCH * qc], vs[:, hj, kt, :],
                        at_back[:, hj, OFF[kt] + lo - klo:OFF[kt] + hi - klo],
                        start=(kt == 0 and hj == 0),
                        stop=(kt == min(TT - 1, 4 * qc + 3) and hj == 1),
                        skip_group_check=True,
                    )

            # ---- all contributions to q-chunk j of the LAST pair ----
            def av2_qc(ytile, j, vs):
                last_kt = min(TT - 1, 4 * j + 3)
                for kt in range(last_kt + 1):
                    klo = 128 * kt
                    lo = max(QCH * j, klo)
                    hi = QCH * j + QCH
                    for hj in range(2):
                        nc.tensor.matmul(
                            ytile[:, lo - QCH * j:hi - QCH * j], vs[:, hj, kt, :],
                            at_back[:, hj, OFF[kt] + lo - klo:OFF[kt] + hi - klo],
                            start=(kt == 0 and hj == 0),
                            stop=(kt == last_kt and hj == 1),
                            skip_group_check=True,
                        )

            with tc.tile_pool(name="psS", bufs=2, space="PSUM") as psS:
                # ---- pair 0 window: S/exp-0 + remaining QKV as filler ----
                with tc.tile_pool(name="psA", bufs=4, space="PSUM") as psA:
                    # prefix ordered by DMA arrival: x chunks 0-1 work first
                    # so the PE isn't parked on the xt2/xt3 transfers, and
                    # every pair-0 Q/K write still precedes the first s_step
                    # (which reads the full T range).
                    for tci in (0, 1):
                        q_group(psA, 0, tci)
                        k_group(psA, 0, tci)
                    v_group(psA, 0)
                    for tci in (2, 3):
                        q_group(psA, 0, tci)
                        k_group(psA, 0, tci)
                    filler = []
                    for et in (1, 2):
                        for tci in range(NQC):
                            filler.append((q_group, et, tci))
                            filler.append((k_group, et, tci))
                    for kt in range(TT):
                        if kt + 1 < TT:
                            v_group(psA, kt + 1)
                        for hj in range(2):
                            s_step(psS, 0, kt, hj)
                        if filler:
                            fn, a1, a2 = filler.pop(0)
                            fn(psA, a1, a2)
                    for fn, a1, a2 in filler:
                        fn(psA, a1, a2)

                # ---- pairs 1,2: S/exp-i + AV drain of pair i-1 ----
                with tc.tile_pool(name="psY", bufs=4, space="PSUM") as psY:
                    def ytile(name):
                        return psY.tile([128, QCH], FP32, tag="y", bufs=4, name=name)

                    # window 1: drain pair 0
                    y0 = [ytile(f"y0_{j}") for j in range(NQC)]
                    for kt in range(TT):
                        for hj in range(2):
                            scale_step(0, kt, hj, vs_bufs[0])
                            av_step(y0, kt, hj, vs_bufs[0])
                            s_step(psS, 1, kt, hj)
                    for j in range(NQC):
                        nc.vector.tensor_copy(y_sb[:, 0, QCH * j:QCH * (j + 1)], y0[j])
                    # window 2: drain pair 1; as each of pair 1's q-chunks
                    # finalizes, recycle its bank for pair 2's same q-chunk
                    # (its exps are complete by then), leaving only q-chunk 3
                    # of pair 2 plus the projection for the tail.
                    y1 = [ytile(f"y1_{j}") for j in range(NQC)]
                    y2 = [None] * NQC
                    for kt in range(TT):
                        for hj in range(2):
                            scale_step(1, kt, hj, vs_bufs[1])
                            av_step(y1, kt, hj, vs_bufs[1])
                            s_step(psS, 2, kt, hj)
                            scale_step(2, kt, hj, vs_bufs[0])
                        if kt % 4 == 3 and kt < TT - 1:
                            j = kt // 4
                            nc.vector.tensor_copy(y_sb[:, 1, QCH * j:QCH * (j + 1)], y1[j])
                            y2[j] = ytile(f"y2_{j}")
                            av2_qc(y2[j], j, vs_bufs[0])
                            nc.vector.tensor_copy(y_sb[:, 2, QCH * j:QCH * (j + 1)], y2[j])
                    nc.vector.tensor_copy(y_sb[:, 1, QCH * 3:], y1[3])
                    # ---- tail: last q-chunk of pair 2, then projection ----
                    y2[3] = ytile("y2_3")
                    av2_qc(y2[3], 3, vs_bufs[0])
                    nc.vector.tensor_copy(y_sb[:, 2, QCH * 3:], y2[3])

                    # ---- output projection (reuses the y bank pool) ----
                    for tt in range(TT):
                        po1 = ytile("po1")
                        po2 = ytile("po2")
                        for et in range(ET):
                            nc.tensor.matmul(
                                po1, y_sb[:, et, tt * 128:(tt + 1) * 128],
                                wp_sb[:, et, 0:QCH],
                                start=(et == 0), stop=(et == ET - 1),
                            )
                            nc.tensor.matmul(
                                po2[:, :C - QCH], y_sb[:, et, tt * 128:(tt + 1) * 128],
                                wp_sb[:, et, QCH:C],
                                start=(et == 0), stop=(et == ET - 1),
                            )
                        o_sb = op.tile([128, C], BF16, tag="o", bufs=3, name="o_sb")
                        nc.vector.tensor_copy(o_sb[:, 0:QCH], po1)
                        nc.vector.tensor_copy(o_sb[:, QCH:C], po2[:, :C - QCH])
                        nc.sync.dma_start(out=out[tt * 128:(tt + 1) * 128, :], in_=o_sb)

    _split_sync_waits(nc)
    return nc


_nc_cache = {}
last_result = None


def kernel(x, w_attn, b_attn, w_proj, b_proj):
    global last_result
    if "nc" not in _nc_cache:
        _nc_cache["nc"] = _build()
    nc = _nc_cache["nc"]

    bf16 = ml_dtypes.bfloat16
    x = np.asarray(x, dtype=np.float32)
    w_attn = np.asarray(w_attn, dtype=np.float32)
    b_attn = np.asarray(b_attn, dtype=np.float32)
    w_proj = np.asarray(w_proj, dtype=np.float32)
    b_proj = np.asarray(b_proj, dtype=np.float32)

    tri = np.where(
        np.arange(128)[None, :] >= np.arange(128)[:, None], 0.0, MASKV
    ).astype(np.float32)

    in_maps = []
    for core in range(NCORES):
        b = core // 2
        e0 = (core % 2) * E
        xt_host = np.ascontiguousarray(
            x[b].T.reshape(CT, 128, NQC, QCH).transpose(2, 1, 0, 3).astype(bf16)
        )
        def _wblk(w):
            return np.ascontiguousarray(
                w.reshape(CT, 128, E).transpose(1, 0, 2).astype(bf16)
            )
        in_maps.append({
            "xT": xt_host,
            "wq": _wblk(w_attn[:, e0:e0 + E]),
            "wk": _wblk(w_attn[:, C + e0:C + e0 + E]),
            "wv": _wblk(w_attn[:, 2 * C + e0:2 * C + e0 + E]),
            "bq": np.ascontiguousarray(b_attn[e0:e0 + E]),
            "bk": np.ascontiguousarray(b_attn[C + e0:C + e0 + E]),
            "bv": np.ascontiguousarray(b_attn[2 * C + e0:2 * C + e0 + E]),
            "wp": np.ascontiguousarray(
                w_proj[e0:e0 + E, :].reshape(ET, 128, C).transpose(1, 0, 2).astype(bf16)
            ),
            "mask": tri,
        })

    trace = os.environ.get("ATT_TRACE", "0")
    kw = {}
    if trace != "0":
        n = min(int(trace), NCORES)
        kw = dict(trace=True, trace_cores=list(range(n)))
    res = run_bass_kernel_spmd(nc, in_maps, list(range(NCORES)), **kw)
    last_result = res

    out = np.zeros((B, T, C), dtype=np.float32)
    for core in range(NCORES):
        out[core // 2] += res.results[core]["out"].astype(np.float32)
    out += b_proj[None, None, :]
    return out


# revision 22
# speedup vs baseline: 1.0948x; 1.0006x over previous
"""Causal self-attention (query-axis softmax) for Trainium2, 8 NeuronCores.

Sharding: 8 cores = 4 batches x 2 half-head-groups. Core c handles batch
c//2 and heads (c%2)*6 .. (c%2)*6+5. Each core computes its heads' full
attention plus its partial output projection; the host sums the two
partials per batch and adds b_proj.

Layout strategy per core (T=2048, C=768, 6 heads, hd=64):
  - host passes x[b].T in bf16 so the QKV contraction dim (C) lands on
    SBUF partitions without any on-chip transpose; all weights bf16.
  - Q,K are produced transposed ([head_d, t]) so S^T = K Q^T tiles have
    softmax's query axis on the free dimension; V is produced in [t, d].
  - softmax over q (free axis): no max-subtraction needed (logits are
    O(1) by construction), exp+rowsum fused on ScalarE via accum_out
    over 1024-wide PSUM chunks; normalization folded into V rows
    (GpSimd scales V[k,:] by 1/denom[k] into zero-padded Vs slots).
  - cross-pair software pipeline: the exp stream (ScalarE) is the
    critical resource, so each head-pair's exp window hides the OTHER
    engines' work: pair 0's window absorbs the remaining QKV matmuls
    (pairs 1-2 Q/K + all V), pair i's window absorbs pair i-1's AV
    drain. exp outputs land in one packed SBUF backlog (at_back);
    AV(kt) for the previous pair re-reads it one step before the
    current pair's exp(kt) overwrites it (WAR dep keeps this safe).
  - causal mask: ragged chunk bounds skip fully-masked blocks; diagonal
    128x128 blocks get a precomputed triangular -30000 add.
  - PSUM: pair-0 window runs QKV accumulators (4 banks) + score chunks
    (4); later windows run score chunks (4) + the previous pair's y^T
    accumulator (4); projection uses its own 4 after scores retire.
"""

import os
import sys

sys.path.insert(0, "/opt/trn_rl_repo")

import numpy as np
import ml_dtypes

import concourse.bass as bass
import concourse.mybir as mybir
import concourse.tile as tile
from concourse.bass_utils import run_bass_kernel_spmd

FP32 = mybir.dt.float32
BF16 = mybir.dt.bfloat16

B, T, C, H = 4, 2048, 768, 12
D = 64                  # head dim
NCORES = 8
HPC = H * B // NCORES   # heads per core = 6
E = HPC * D             # qkv slice width per core = 384
CT = C // 128           # c tiles = 6
ET = E // 128           # e tiles = 3
TT = T // 128           # t tiles = 16
QCH = 512               # fp32 matmul moving chunk (PSUM bank limit)
NQC = T // QCH          # 4
BCH = 1024              # exp chunk
NBC = T // BCH          # 2
MASKV = -30000.0
SCALE = 1.0 / 8.0       # 1/sqrt(hd)
Exp = mybir.ActivationFunctionType.Exp

# packed at_back offsets: slot kt holds columns [klo, T) of the exp tile
OFF = [0] * (TT + 1)
for _kt in range(TT):
    OFF[_kt + 1] = OFF[_kt] + (T - 128 * _kt)
TOFF = OFF[TT]          # 17408


def _split_sync_waits(nc):
    """This container's walrus encodes at most one sync wait per
    instruction for several instruction structs; hoist extra waits onto
    same-engine nops placed immediately before the instruction."""
    for f in nc.m.functions:
        for bb in f.blocks:
            new_insts = []
            for inst in bb.instructions:
                si = inst.sync_info
                waits = list(si.on_wait) if si is not None and si.on_wait else []
                if len(waits) > 1:
                    for w in waits[:-1]:
                        nop = mybir.InstNoOp(
                            name=nc.get_next_instruction_name(),
                            engine=inst.engine,
                            sync_info=mybir.SyncInfo(on_wait=[w], on_update=[]),
                            bass_nofuse=True,
                        )
                        nc.register_instruction(nop)
                        new_insts.append(nop)
                    inst.sync_info = mybir.SyncInfo(
                        on_wait=[waits[-1]], on_update=list(si.on_update or [])
                    )
                new_insts.append(inst)
            bb.instructions[:] = new_insts


def _build():
    nc = bass.Bass("TRN2")
    xT = nc.dram_tensor("xT", [NQC, 128, CT, QCH], BF16, kind="ExternalInput")
    wq = nc.dram_tensor("wq", [128, CT, E], BF16, kind="ExternalInput")
    wk = nc.dram_tensor("wk", [128, CT, E], BF16, kind="ExternalInput")
    wv = nc.dram_tensor("wv", [128, CT, E], BF16, kind="ExternalInput")
    bq = nc.dram_tensor("bq", [E], FP32, kind="ExternalInput")
    bk = nc.dram_tensor("bk", [E], FP32, kind="ExternalInput")
    bv = nc.dram_tensor("bv", [E], FP32, kind="ExternalInput")
    wp = nc.dram_tensor("wp", [128, ET, C], BF16, kind="ExternalInput")
    mask = nc.dram_tensor("mask", [128, 128], FP32, kind="ExternalInput")
    out = nc.dram_tensor("out", [T, C], BF16, kind="ExternalOutput")

    with tile.TileContext(nc) as tc:
        with (
            tc.tile_pool(name="wts", bufs=1) as wts,
            tc.tile_pool(name="big", bufs=1) as big,
            tc.tile_pool(name="sm", bufs=4) as sm,
            tc.tile_pool(name="op", bufs=3) as op,
        ):
            # ---- constant loads: few big DMAs (each dma_start has ~1us
            # issue overhead on the sync engine), ordered for earliest
            # compute start ----
            xt_all = wts.tile([128, NQC, CT, QCH], BF16, name="xt_all")
            wq_sb = wts.tile([128, CT, E], BF16)
            wk_sb = wts.tile([128, CT, E], BF16)
            wv_sb = wts.tile([128, CT, E], BF16)
            bq_sb = wts.tile([128, ET], FP32)
            bk_sb = wts.tile([128, ET], FP32)
            mask_sb = wts.tile([128, 128], FP32)
            nc.sync.dma_start(out=xt_all[:, 0], in_=xT[0])
            nc.sync.dma_start(out=wq_sb, in_=wq[:])
            nc.sync.dma_start(out=bq_sb, in_=bq.rearrange("(et p) -> p et", p=128))
            nc.sync.dma_start(out=mask_sb, in_=mask[:])
            nc.sync.dma_start(out=wk_sb, in_=wk[:])
            nc.sync.dma_start(out=bk_sb, in_=bk.rearrange("(et p) -> p et", p=128))
            nc.sync.dma_start(out=xt_all[:, 1], in_=xT[1])
            nc.sync.dma_start(out=xt_all[:, 2], in_=xT[2])
            nc.sync.dma_start(out=xt_all[:, 3], in_=xT[3])
            nc.sync.dma_start(out=wv_sb, in_=wv[:])
            bv_sb = wts.tile([128, E], FP32)
            nc.sync.dma_start(out=bv_sb, in_=bv[None, :].to_broadcast((128, E)))
            wp_sb = wts.tile([128, ET, C], BF16)
            nc.sync.dma_start(out=wp_sb, in_=wp[:])

            qt2 = big.tile([128, ET, T], BF16)      # [d-in-pair, pair, t]
            ktp2 = big.tile([128, ET, 2, T], BF16)  # [d(+zero half), pair, head-in-pair, t]
            v_sb = big.tile([128, TT, E], BF16)     # [t-in-tile, ttile, (head,d)]
            y_sb = big.tile([128, ET, T], BF16)     # [hd-in-pair, pair, t]
            at_back = big.tile([128, 2, TOFF], BF16)  # packed exp backlog per hj
            rcp_all = big.tile([128, 2, TT], FP32)  # 1/denom per (hj, kt), current pair
            # zero-fill ktp2 per-pair on DVE (gpsimd's startup preamble is
            # ~10us; pair 0's K bias adds must unblock early)
            for et in range(ET):
                nc.vector.memset(ktp2[:, et].bitcast(mybir.dt.uint16), 0)
            # two rotating scaled-V buffers (pairs alternate); slot (hj,kt)
            # holds Vs in cols hj*64..hj*64+64, zeros elsewhere (so two
            # heads' AV matmuls can accumulate into one shared y^T psum).
            vs_bufs = [
                big.tile([128, 2, TT, 128], BF16, name=f"vs{i}") for i in range(2)
            ]
            for t_ in vs_bufs:
                nc.gpsimd.memset(t_, 0.0)

            # ---- QKV group emitters ----
            def q_group(psA, et, tci):
                cols = slice(tci * QCH, (tci + 1) * QCH)
                pq = psA.tile([128, QCH], FP32, tag="ps", bufs=4, name="pq")
                for ct in range(CT):
                    nc.tensor.matmul(
                        pq, wq_sb[:, ct, et * 128:(et + 1) * 128],
                        xt_all[:, tci, ct, :],
                        start=(ct == 0), stop=(ct == CT - 1),
                    )
                nc.vector.tensor_scalar_add(qt2[:, et, cols], pq, bq_sb[:, et:et + 1])

            def k_group(psA, et, tci):
                cols = slice(tci * QCH, (tci + 1) * QCH)
                pk = psA.tile([128, QCH], FP32, tag="ps", bufs=4, name="pk")
                for ct in range(CT):
                    nc.tensor.matmul(
                        pk, wk_sb[:, ct, et * 128:(et + 1) * 128],
                        xt_all[:, tci, ct, :],
                        start=(ct == 0), stop=(ct == CT - 1),
                    )
                nc.vector.tensor_scalar_add(
                    ktp2[0:64, et, 0, cols], pk[0:64, :], bk_sb[0:64, et:et + 1]
                )
                nc.vector.tensor_scalar_add(
                    ktp2[64:128, et, 1, cols], pk[64:128, :], bk_sb[64:128, et:et + 1]
                )

            def v_group(psA, tt):
                tci, ttl = tt // 4, tt % 4
                pv = psA.tile([128, QCH], FP32, tag="ps", bufs=4, name="pv")
                for ct in range(CT):
                    nc.tensor.matmul(
                        pv[:, :E], xt_all[:, tci, ct, ttl * 128:(ttl + 1) * 128],
                        wv_sb[:, ct, :],
                        start=(ct == 0), stop=(ct == CT - 1),
                    )
                nc.vector.tensor_add(v_sb[:, tt, :], pv[:, :E], bv_sb)

            # ---- one S^T/exp step; 1/denom lands in rcp_all ----
            # dve_sum: rowsum via a DVE reduce over the exp output instead
            # of the ScalarE accumulator (ScalarE is the critical engine in
            # the steady-state windows; pair 0's window is PE-bound and its
            # DVE is loaded with QKV evacuations, so it keeps ACT accum).
            def s_step(psS, hp, kt, hj):
                klo = 128 * kt
                bc0 = klo // BCH
                dve_sum = False  # DVE reduce costs more than ACT accum saves
                sums = sm.tile([128, NBC], FP32, tag="sums", bufs=4, name="sums")
                for bc in range(bc0, NBC):
                    blo = max(BCH * bc, klo)
                    s_ps = psS.tile([128, BCH], FP32, tag="s", bufs=2, name="s_ps")
                    for half in range(2):
                        plo = max(blo, BCH * bc + half * QCH)
                        phi = BCH * bc + (half + 1) * QCH
                        if plo >= phi:
                            continue
                        nc.tensor.matmul(
                            s_ps[:, plo - BCH * bc:phi - BCH * bc],
                            ktp2[:, hp, hj, klo:klo + 128],
                            qt2[:, hp, plo:phi],
                            start=True, stop=True,
                        )
                    if bc == bc0:
                        off = klo - BCH * bc
                        nc.vector.tensor_add(
                            s_ps[:, off:off + 128], s_ps[:, off:off + 128], mask_sb
                        )
                    nc.scalar.activation(
                        at_back[:, hj, OFF[kt] + blo - klo:OFF[kt] + BCH * (bc + 1) - klo],
                        s_ps[:, blo - BCH * bc:],
                        Exp, scale=SCALE,
                        accum_out=None if dve_sum else sums[:, bc:bc + 1],
                    )
                if dve_sum:
                    stot = sm.tile([128, 1], FP32, tag="stot", bufs=4, name="stot")
                    nc.vector.reduce_sum(
                        stot, at_back[:, hj, OFF[kt]:OFF[kt] + T - klo],
                        axis=mybir.AxisListType.X,
                    )
                    nc.vector.reciprocal(rcp_all[:, hj, kt:kt + 1], stot)
                elif bc0 == NBC - 1:
                    nc.vector.reciprocal(rcp_all[:, hj, kt:kt + 1], sums[:, bc0:NBC])
                else:
                    stot = sm.tile([128, 1], FP32, tag="stot", bufs=4, name="stot")
                    nc.vector.reduce_sum(stot, sums[:, bc0:NBC], axis=mybir.AxisListType.X)
                    nc.vector.reciprocal(rcp_all[:, hj, kt:kt + 1], stot)

            # ---- scale V rows of the PREVIOUS pair by its 1/denom ----
            def scale_step(hp_prev, kt, hj, vs):
                hl = 2 * hp_prev + hj
                nc.vector.tensor_scalar_mul(
                    vs[:, hj, kt, hj * 64:hj * 64 + 64],
                    v_sb[:, kt, hl * 64:(hl + 1) * 64], rcp_all[:, hj, kt:kt + 1]
                )

            # ---- AV drain for one (kt, hj) of the previous pair; y^T is
            # held as four 512-wide 1-bank tiles so finished q-chunks can
            # release their bank mid-window ----
            def av_step(ytiles, kt, hj, vs):
                klo = 128 * kt
                for qc in range(kt // 4, NQC):
                    lo = max(QCH * qc, klo)
                    hi = QCH * qc + QCH
                    nc.tensor.matmul(
                        ytiles[qc][:, lo - QCH * qc:hi - QCH * qc], vs[:, hj, kt, :],
                        at_back[:, hj, OFF[kt] + lo - klo:OFF[kt] + hi - klo],
                        start=(kt == 0 and hj == 0),
                        stop=(kt == min(TT - 1, 4 * qc + 3) and hj == 1),
                        skip_group_check=True,
                    )

            # ---- all contributions to q-chunk j of the LAST pair ----
            def av2_qc(ytile, j, vs):
                last_kt = min(TT - 1, 4 * j + 3)
                for kt in range(last_kt + 1):
                    klo = 128 * kt
                    lo = max(QCH * j, klo)
                    hi = QCH * j + QCH
                    for hj in range(2):
                        nc.tensor.matmul(
                            ytile[:, lo - QCH * j:hi - QCH * j], vs[:, hj, kt, :],
                            at_back[:, hj, OFF[kt] + lo - klo:OFF[kt] + hi - klo],
                            start=(kt == 0 and hj == 0),
                            stop=(kt == last_kt and hj == 1),
                            skip_group_check=True,
                        )

            with tc.tile_pool(name="psS", bufs=2, space="PSUM") as psS:
                # ---- pair 0 window: S/exp-0 + remaining QKV as filler ----
                with tc.tile_pool(name="psA", bufs=4, space="PSUM") as psA:
                    # prefix ordered by DMA arrival: x chunks 0-1 work first
                    # so the PE isn't parked on the xt2/xt3 transfers, and
                    # every pair-0 Q/K write still precedes the first s_step
                    # (which reads the full T range).
                    for tci in (0, 1):
                        q_group(psA, 0, tci)
                        k_group(psA, 0, tci)
                    v_group(psA, 0)
                    for tci in (2, 3):
                        q_group(psA, 0, tci)
                        k_group(psA, 0, tci)
                    filler = []
                    for et in (1, 2):
                        for tci in range(NQC):
                            filler.append((q_group, et, tci))
                            filler.append((k_group, et, tci))
                    for kt in range(TT):
                        if kt + 1 < TT:
                            v_group(psA, kt + 1)
                        for hj in range(2):
                            s_step(psS, 0, kt, hj)
                        if filler:
                            fn, a1, a2 = filler.pop(0)
                            fn(psA, a1, a2)
                    for fn, a1, a2 in filler:
                        fn(psA, a1, a2)

                # ---- pairs 1,2: S/exp-i + AV drain of pair i-1 ----
                with tc.tile_pool(name="psY", bufs=4, space="PSUM") as psY:
                    def ytile(name):
                        return psY.tile([128, QCH], FP32, tag="y", bufs=4, name=name)

                    # window 1: drain pair 0
                    y0 = [ytile(f"y0_{j}") for j in range(NQC)]
                    for kt in range(TT):
                        for hj in range(2):
                            scale_step(0, kt, hj, vs_bufs[0])
                            av_step(y0, kt, hj, vs_bufs[0])
                            s_step(psS, 1, kt, hj)
                    for j in range(NQC):
                        nc.vector.tensor_copy(y_sb[:, 0, QCH * j:QCH * (j + 1)], y0[j])
                    # window 2: drain pair 1; as each of pair 1's q-chunks
                    # finalizes, recycle its bank for pair 2's same q-chunk
                    # (its exps are complete by then), leaving only q-chunk 3
                    # of pair 2 plus the projection for the tail.
                    y1 = [ytile(f"y1_{j}") for j in range(NQC)]
                    y2 = [None] * NQC
                    for kt in range(TT):
                        for hj in range(2):
                            scale_step(1, kt, hj, vs_bufs[1])
                            av_step(y1, kt, hj, vs_bufs[1])
                            s_step(psS, 2, kt, hj)
                            scale_step(2, kt, hj, vs_bufs[0])
                        if kt % 4 == 3 and kt < TT - 1:
                            j = kt // 4
                            nc.vector.tensor_copy(y_sb[:, 1, QCH * j:QCH * (j + 1)], y1[j])
                            y2[j] = ytile(f"y2_{j}")
                            av2_qc(y2[j], j, vs_bufs[0])
                            nc.vector.tensor_copy(y_sb[:, 2, QCH * j:QCH * (j + 1)], y2[j])
                    nc.vector.tensor_copy(y_sb[:, 1, QCH * 3:], y1[3])
                    # ---- tail: last q-chunk of pair 2, then projection ----
                    y2[3] = ytile("y2_3")
                    av2_qc(y2[3], 3, vs_bufs[0])
                    nc.vector.tensor_copy(y_sb[:, 2, QCH * 3:], y2[3])

                    # ---- output projection (reuses the y bank pool) ----
                    for tt in range(TT):
                        po1 = ytile("po1")
                        po2 = ytile("po2")
                        for et in range(ET):
                            nc.tensor.matmul(
                                po1, y_sb[:, et, tt * 128:(tt + 1) * 128],
                                wp_sb[:, et, 0:QCH],
                                start=(et == 0), stop=(et == ET - 1),
                            )
                            nc.tensor.matmul(
                                po2[:, :C - QCH], y_sb[:, et, tt * 128:(tt + 1) * 128],
                                wp_sb[:, et, QCH:C],
                                start=(et == 0), stop=(et == ET - 1),
                            )
                        o_sb = op.tile([128, C], BF16, tag="o", bufs=3, name="o_sb")
                        nc.vector.tensor_copy(o_sb[:, 0:QCH], po1)
                        nc.vector.tensor_copy(o_sb[:, QCH:C], po2[:, :C - QCH])
                        # alternate output DGE queues so the last tiles'
                        # writebacks drain in parallel (ScalarE idles here)
                        eng = nc.sync if tt % 2 == 0 else nc.scalar
                        eng.dma_start(out=out[tt * 128:(tt + 1) * 128, :], in_=o_sb)

    _split_sync_waits(nc)
    return nc


_nc_cache = {}
last_result = None


def kernel(x, w_attn, b_attn, w_proj, b_proj):
    global last_result
    if "nc" not in _nc_cache:
        _nc_cache["nc"] = _build()
    nc = _nc_cache["nc"]

    bf16 = ml_dtypes.bfloat16
    x = np.asarray(x, dtype=np.float32)
    w_attn = np.asarray(w_attn, dtype=np.float32)
    b_attn = np.asarray(b_attn, dtype=np.float32)
    w_proj = np.asarray(w_proj, dtype=np.float32)
    b_proj = np.asarray(b_proj, dtype=np.float32)

    tri = np.where(
        np.arange(128)[None, :] >= np.arange(128)[:, None], 0.0, MASKV
    ).astype(np.float32)

    in_maps = []
    for core in range(NCORES):
        b = core // 2
        e0 = (core % 2) * E
        xt_host = np.ascontiguousarray(
            x[b].T.reshape(CT, 128, NQC, QCH).transpose(2, 1, 0, 3).astype(bf16)
        )
        def _wblk(w):
            return np.ascontiguousarray(
                w.reshape(CT, 128, E).transpose(1, 0, 2).astype(bf16)
            )
        in_maps.append({
            "xT": xt_host,
            "wq": _wblk(w_attn[:, e0:e0 + E]),
            "wk": _wblk(w_attn[:, C + e0:C + e0 + E]),
            "wv": _wblk(w_attn[:, 2 * C + e0:2 * C + e0 + E]),
            "bq": np.ascontiguousarray(b_attn[e0:e0 + E]),
            "bk": np.ascontiguousarray(b_attn[C + e0:C + e0 + E]),
            "bv": np.ascontiguousarray(b_attn[2 * C + e0:2 * C + e0 + E]),
            "wp": np.ascontiguousarray(
                w_proj[e0:e0 + E, :].reshape(ET, 128, C).transpose(1, 0, 2).astype(bf16)
            ),
            "mask": tri,
        })

    trace = os.environ.get("ATT_TRACE", "0")
    kw = {}
    if trace != "0":
        n = min(int(trace), NCORES)
        kw = dict(trace=True, trace_cores=list(range(n)))
    res = run_bass_kernel_spmd(nc, in_maps, list(range(NCORES)), **kw)
    last_result = res

    out = np.zeros((B, T, C), dtype=np.float32)
    for core in range(NCORES):
        out[core // 2] += res.results[core]["out"].astype(np.float32)
    out += b_proj[None, None, :]
    return out
